# revision 19
# baseline (speedup 1.0000x reference)
"""GAT (2-layer graph attention network) on 8 Trainium2 NeuronCores.

Strategy: partition nodes (and incident edges, grouped by destination) across
the 8 cores; replicate the small weight matrices; all-gather node features
between layers. Per-edge rows come from per-node DRAM tables via batched
dma_gather (one instruction per ~1-1.3k edges; int16 index range handled by
splitting each block's slots into low/high node-id groups gathered from two
base views). Segment softmax reduce uses PSUM-accumulated matmuls against
one-hot scatter masks generated ON-CHIP (iota + is_equal); the dst->edge
expansion mask comes from a DMA-replicated mrow row + per-partition iota.
Softmax max-subtraction is skipped (mathematically identical, and verified
numerically safe for this model's logit range).
"""
import numpy as np
import ml_dtypes

import concourse.bass as bass
import concourse.bacc as bacc
import concourse.mybir as mybir
import concourse.tile as tile
from concourse.bass_utils import run_bass_kernel_spmd
from concourse.masks import make_identity

P = 128
NCORES = 8
N = 50000
F_IN = 165
D = 64
H1 = 4
NSH = N // NCORES            # 6250 nodes per core
NBLK = 49                    # dst blocks of 128 (6272 slots)
NPAD = NBLK * P              # 6272
C1 = 66                      # cols per head in T1: 64 h1 | a_s | 1.0
R1 = H1 * C1                 # 264
R2 = C1                      # 66 (single head)
W1ROW = 384                  # T1 row width (264 used, 768B rows)
W2ROW = 128                  # T2 row width (66 used, 256B rows)
T1_ROWS = N + 2              # sentinel rows at 0 and N+1; node n -> row n+1
T2_ROWS = NCORES * NPAD + 2  # padded shard layout + sentinels
I16MAX = 32767               # dma_gather idx range per view
HIBASE1 = T1_ROWS - I16MAX   # high-view base (low/high views overlap)
HIBASE2 = T2_ROWS - I16MAX
LOWN = 1024                  # steered low-group size (2x512 gathers, no pads)
PADMROW = 200.0              # mrow for invalid slots (matches no one-hot column)


def _units(k):
    """Decompose k chunks into dma_gather units of 4/2/1 chunks (512/256/128)."""
    u = [4] * (k // 4)
    r = k % 4
    if r == 3:
        u += [2, 1]
    elif r:
        u += [r]
    return u
NEG = -1.0e30

f32 = mybir.dt.float32
bf16 = mybir.dt.bfloat16
i16 = mybir.dt.int16
AF = mybir.ActivationFunctionType
OP = mybir.AluOpType


# ----------------------------------------------------------------------------
# Host-side preparation
# ----------------------------------------------------------------------------

def prep_weights(inp):
    """Fold biases and attention vectors into augmented weight matrices."""
    W1 = np.asarray(inp["W1"], np.float32)          # [64, 256]
    W2 = np.asarray(inp["W2"], np.float32)          # [256, 64]
    as1 = np.asarray(inp["att_src1"], np.float32)   # [4, 64]
    ad1 = np.asarray(inp["att_dst1"], np.float32)
    as2 = np.asarray(inp["att_src2"], np.float32)   # [1, 64]
    ad2 = np.asarray(inp["att_dst2"], np.float32)

    projW = np.concatenate([np.asarray(inp["proj_W"], np.float32),
                            np.asarray(inp["proj_b"], np.float32)[None, :]], 0)  # [166, 64]

    W1p = np.zeros((D + 1, R1 + 4), np.float32)     # [65, 268]
    for h in range(H1):
        Wh = W1[:, h * D:(h + 1) * D]
        W1p[:D, h * C1:h * C1 + D] = Wh
        W1p[:D, h * C1 + D] = Wh @ as1[h]
        W1p[D, h * C1 + D + 1] = 1.0
        W1p[:D, R1 + h] = Wh @ ad1[h]

    W2p = np.zeros((H1 * D + 1, 68), np.float32)    # [257, 68]
    W2p[:H1 * D, 0:D] = W2
    W2p[:H1 * D, D] = W2 @ as2[0]
    W2p[H1 * D, D + 1] = 1.0
    W2p[:H1 * D, D + 2] = W2 @ ad2[0]

    clsp = np.concatenate([np.asarray(inp["cls_W"], np.float32),
                           np.asarray(inp["cls_b"], np.float32)[None, :]], 0)  # [65, 1]
    return {
        "projW_a": projW[:128], "projW_b": projW[128:],         # [128,64],[38,64]
        "W1p": W1p,
        "W2p_a": W2p[:128], "W2p_b": W2p[128:256], "W2p_c": W2p[256:257],
        "clsp": clsp,
        "b1_row": np.asarray(inp["b1"], np.float32)[None, :],
        "b2_row": np.asarray(inp["b2"], np.float32)[None, :],
        "iota_row": np.arange(P, dtype=np.float32)[None, :].astype(ml_dtypes.bfloat16),
        "iota_col": np.arange(P, dtype=np.float32)[:, None].astype(ml_dtypes.bfloat16),
    }


def _wrap16(a):
    """idx list [n] -> [128, n//16] int16 tile (i at [i%16, i//16], x8 replicas)."""
    w = np.ascontiguousarray(a.reshape(-1, 16).T).astype(np.int16)
    return np.tile(w, (8, 1))


def prep_edges(edge_index):
    """Partition/sort/pack edges. Returns per-core dict + (klo1,khi1,klo2,khi2)."""
    ei = np.asarray(edge_index).astype(np.int64)
    loop = np.arange(N, dtype=np.int64)
    src = np.concatenate([ei[0], loop])
    dst = np.concatenate([ei[1], loop])

    per_core = []
    for c in range(NCORES):
        sel = (dst >= c * NSH) & (dst < (c + 1) * NSH)
        s = src[sel]
        ld = dst[sel] - c * NSH
        order = np.argsort(ld, kind="stable")
        s, ld = s[order], ld[order]
        per_core.append((s, ld))

    # per-layer keys (table row = key; row 0 / last are sentinels)
    def keys(s, layer):
        if layer == 1:
            return s + 1
        q = (s // NSH) * NPAD + s % NSH
        return q + 1

    # Per block, keys < HIBASE must go low, keys >= I16MAX must go high;
    # keys in [HIBASE, I16MAX) are steered to fill the low group to exactly
    # LOWN slots. klo is then fixed at LOWN/128; khi is the global max.
    klo1 = klo2 = LOWN // P
    khi = [0, 0]
    assign = []            # per core, per layer: boolean to-low mask per edge
    for c in range(NCORES):
        s, ld = per_core[c]
        blk = ld // P
        core_assign = {}
        for L in (1, 2):
            hibase = HIBASE1 if L == 1 else HIBASE2
            k = keys(s, L)
            tolow = np.zeros(len(k), bool)
            for b in range(NBLK):
                m = np.where(blk == b)[0]
                kb = k[m]
                forced_low = kb < hibase
                nfl = int(forced_low.sum())
                assert nfl <= LOWN, f"block {b} core {c} forced-low {nfl} > {LOWN}"
                tolow[m[forced_low]] = True
                free = m[(kb >= hibase) & (kb < I16MAX)]
                take = min(LOWN - nfl, len(free))
                tolow[free[:take]] = True
                nh = len(m) - nfl - take
                khi[L - 1] = max(khi[L - 1], int(np.ceil(nh / P)))
            core_assign[L] = tolow
        assign.append(core_assign)
    khi1, khi2 = khi[0], khi[1]
    cpb1, cpb2 = klo1 + khi1, klo2 + khi2

    cores = []
    for c in range(NCORES):
        s, ld = per_core[c]
        blk = ld // P
        mrow_all = ld % P
        d = {}
        for L, (kl, kh, nrows, hibase) in (
                (1, (klo1, khi1, T1_ROWS, HIBASE1)),
                (2, (klo2, khi2, T2_ROWS, HIBASE2))):
            cpb = kl + kh
            pad_hi = nrows - 1 - hibase                          # last sentinel
            k = keys(s, L)
            tolow = assign[c][L]
            idxlo = np.zeros((NBLK, kl * P), np.int64)           # pad -> sentinel row 0
            idxhi = np.full((NBLK, kh * P), pad_hi, np.int64)
            mrow = np.full((NBLK, cpb * P), PADMROW, np.float64)
            for b in range(NBLK):
                m = blk == b
                lowm = m & tolow
                highm = m & ~tolow
                nl, nh = int(lowm.sum()), int(highm.sum())
                idxlo[b, :nl] = k[lowm]
                idxhi[b, :nh] = k[highm] - hibase
                mrow[b, :nl] = mrow_all[lowm]
                mrow[b, kl * P:kl * P + nh] = mrow_all[highm]
            d[f"idxlo{L}"] = np.concatenate(
                [_wrap16(idxlo[b]) for b in range(NBLK)], axis=1)
            d[f"idxhi{L}"] = np.concatenate(
                [_wrap16(idxhi[b]) for b in range(NBLK)], axis=1)
            # mrowb[e, b*cpb + kk] = mrow of slot (b, kk, e)
            mr = mrow.reshape(NBLK, cpb, P)
            d[f"mrowb{L}"] = np.ascontiguousarray(
                mr.transpose(2, 0, 1).reshape(P, NBLK * cpb)).astype(ml_dtypes.bfloat16)
            d[f"mrowT{L}"] = mrow.astype(ml_dtypes.bfloat16)    # [NBLK, cpb*P]
        cores.append(d)
    return cores, (klo1, khi1, klo2, khi2)


# ----------------------------------------------------------------------------
# Device program
# ----------------------------------------------------------------------------

def n_strip_chunks():
    """6250 = 50 x 125 node chunks for strip passes."""
    return [(j * 125, 125) for j in range(50)]


def build_program(ks, reps=1, ablate=()):
    klo1, khi1, klo2, khi2 = ks
    cpb1, cpb2 = klo1 + khi1, klo2 + khi2
    nc = bacc.Bacc(None, num_devices=NCORES, dynamic_dma_scratch_size=49152)

    x_in = nc.declare_dram_parameter("x_strip", [NSH, F_IN], f32, isOutput=False)
    il1_in = nc.declare_dram_parameter("idxlo1", [P, NBLK * klo1 * 8], i16, isOutput=False)
    ih1_in = nc.declare_dram_parameter("idxhi1", [P, NBLK * khi1 * 8], i16, isOutput=False)
    il2_in = nc.declare_dram_parameter("idxlo2", [P, NBLK * klo2 * 8], i16, isOutput=False)
    ih2_in = nc.declare_dram_parameter("idxhi2", [P, NBLK * khi2 * 8], i16, isOutput=False)
    mb1_in = nc.declare_dram_parameter("mrowb1", [P, NBLK * cpb1], bf16, isOutput=False)
    mb2_in = nc.declare_dram_parameter("mrowb2", [P, NBLK * cpb2], bf16, isOutput=False)
    mt1_in = nc.declare_dram_parameter("mrowT1", [NBLK, cpb1 * P], bf16, isOutput=False)
    mt2_in = nc.declare_dram_parameter("mrowT2", [NBLK, cpb2 * P], bf16, isOutput=False)
    pwa_in = nc.declare_dram_parameter("projW_a", [128, D], f32, isOutput=False)
    pwb_in = nc.declare_dram_parameter("projW_b", [38, D], f32, isOutput=False)
    w1p_in = nc.declare_dram_parameter("W1p", [D + 1, R1 + 4], f32, isOutput=False)
    w2pa_in = nc.declare_dram_parameter("W2p_a", [128, 68], f32, isOutput=False)
    w2pb_in = nc.declare_dram_parameter("W2p_b", [128, 68], f32, isOutput=False)
    w2pc_in = nc.declare_dram_parameter("W2p_c", [1, 68], f32, isOutput=False)
    cls_in = nc.declare_dram_parameter("clsp", [D + 1, 1], f32, isOutput=False)
    b1_in = nc.declare_dram_parameter("b1_row", [1, H1 * D], f32, isOutput=False)
    b2_in = nc.declare_dram_parameter("b2_row", [1, D], f32, isOutput=False)
    ior_in = nc.declare_dram_parameter("iota_row", [1, P], bf16, isOutput=False)
    ioc_in = nc.declare_dram_parameter("iota_col", [P, 1], bf16, isOutput=False)
    y_out = nc.declare_dram_parameter("y", [NPAD], f32, isOutput=True)

    # internal DRAM
    T1 = nc.dram_tensor("T1", [T1_ROWS, W1ROW], bf16)
    T2 = nc.dram_tensor("T2", [T2_ROWS, W2ROW], bf16)
    ad1_d = nc.dram_tensor("ad1", [NPAD, H1], bf16)
    h1sh = nc.dram_tensor("h1sh", [D + 1, NSH], bf16)
    h1full = nc.dram_tensor("h1full", [NCORES, D + 1, NSH], bf16, addr_space="Shared")
    h2sh = nc.dram_tensor("h2sh", [2 * P + 1, NPAD], bf16)
    h2full = nc.dram_tensor("h2full", [NCORES, 2 * P + 1, NPAD], bf16, addr_space="Shared")

    import contextlib
    def rep_ctx():
        return tc.For_i(0, reps, 1) if reps > 1 else contextlib.nullcontext()

    with tile.TileContext(nc) as tc:
        with tc.tile_pool(name="const", bufs=1) as cpool:
            ident = cpool.tile([P, P], f32)
            make_identity(nc, ident[:])
            pwa = cpool.tile([128, D], f32)
            nc.sync.dma_start(out=pwa[:], in_=pwa_in[:])
            pwb = cpool.tile([38, D], f32)
            nc.sync.dma_start(out=pwb[:], in_=pwb_in[:])
            w1p = cpool.tile([D + 1, R1 + 4], f32)
            nc.sync.dma_start(out=w1p[:], in_=w1p_in[:])
            w2pa = cpool.tile([128, 68], f32)
            nc.sync.dma_start(out=w2pa[:], in_=w2pa_in[:])
            w2pb = cpool.tile([128, 68], f32)
            nc.sync.dma_start(out=w2pb[:], in_=w2pb_in[:])
            w2pc = cpool.tile([1, 68], f32)
            nc.sync.dma_start(out=w2pc[:], in_=w2pc_in[:])
            clsp = cpool.tile([D + 1, 1], f32)
            nc.sync.dma_start(out=clsp[:], in_=cls_in[:])
            w1p_bf = cpool.tile([D + 1, R1 + 4], bf16)
            nc.vector.tensor_copy(out=w1p_bf[:], in_=w1p[:])
            w2pa_bf = cpool.tile([128, 68], bf16)
            nc.vector.tensor_copy(out=w2pa_bf[:], in_=w2pa[:])
            w2pb_bf = cpool.tile([128, 68], bf16)
            nc.vector.tensor_copy(out=w2pb_bf[:], in_=w2pb[:])
            w2pc_bf = cpool.tile([1, 68], bf16)
            nc.vector.tensor_copy(out=w2pc_bf[:], in_=w2pc[:])

            il1 = cpool.tile([P, NBLK * klo1 * 8], i16)
            nc.sync.dma_start(out=il1[:], in_=il1_in[:])
            ih1 = cpool.tile([P, NBLK * khi1 * 8], i16)
            nc.sync.dma_start(out=ih1[:], in_=ih1_in[:])
            il2 = cpool.tile([P, NBLK * klo2 * 8], i16)
            nc.sync.dma_start(out=il2[:], in_=il2_in[:])
            ih2 = cpool.tile([P, NBLK * khi2 * 8], i16)
            nc.sync.dma_start(out=ih2[:], in_=ih2_in[:])
            mrowb1 = cpool.tile([P, NBLK * cpb1], bf16)
            nc.sync.dma_start(out=mrowb1[:], in_=mb1_in[:])
            mrowb2 = cpool.tile([P, NBLK * cpb2], bf16)
            nc.sync.dma_start(out=mrowb2[:], in_=mb2_in[:])

            # iota constants: row (every partition 0..127) and column (partition
            # idx). Host-provided: gpsimd.iota (standard Q7 lib) cannot coexist
            # with dma_gather (mlp Q7 lib) in one program.
            iota_r = cpool.tile([P, P], bf16)
            nc.sync.dma_start(out=iota_r[:], in_=ior_in[:].to_broadcast([P, P]))
            iota_c = cpool.tile([P, 1], bf16)
            nc.sync.dma_start(out=iota_c[:], in_=ioc_in[:])

            # bias tiles broadcast to 128 partitions via K=1 matmul
            ones1 = cpool.tile([1, P], f32)
            nc.vector.memset(ones1[:], 1.0)
            b1row = cpool.tile([1, H1 * D], f32)
            nc.sync.dma_start(out=b1row[:], in_=b1_in[:])
            b2row = cpool.tile([1, D], f32)
            nc.sync.dma_start(out=b2row[:], in_=b2_in[:])
            b1t = cpool.tile([P, H1 * D], f32)
            b2t = cpool.tile([P, D], f32)
            with tc.tile_pool(name="pbias", bufs=1, space="PSUM") as pb:
                bp1 = pb.tile([P, H1 * D], f32)
                nc.tensor.matmul(out=bp1[:], lhsT=ones1[:], rhs=b1row[:], start=True, stop=True)
                nc.vector.tensor_copy(out=b1t[:], in_=bp1[:])
                bp2 = pb.tile([P, D], f32)
                nc.tensor.matmul(out=bp2[:], lhsT=ones1[:], rhs=b2row[:], start=True, stop=True)
                nc.vector.tensor_copy(out=b2t[:], in_=bp2[:])

            # sentinel rows (full table width; a_s cols = NEG -> w = 0)
            sent1 = cpool.tile([1, W1ROW], bf16)
            nc.vector.memset(sent1[:], 0.0)
            nc.vector.memset(sent1[0:1, D:D + (H1 - 1) * C1 + 1:C1], NEG)
            nc.sync.dma_start(out=T1[0:1, :], in_=sent1[:])
            nc.sync.dma_start(out=T1[T1_ROWS - 1:T1_ROWS, :], in_=sent1[:])
            sent2 = cpool.tile([1, W2ROW], bf16)
            nc.vector.memset(sent2[:], 0.0)
            nc.vector.memset(sent2[0:1, D:D + 1], NEG)
            nc.sync.dma_start(out=T2[0:1, :], in_=sent2[:])
            nc.sync.dma_start(out=T2[T2_ROWS - 1:T2_ROWS, :], in_=sent2[:])
            # a_d pad rows
            zpad = cpool.tile([NPAD - NSH, H1], bf16)
            nc.vector.memset(zpad[:], 0.0)
            nc.sync.dma_start(out=ad1_d[NSH:NPAD, :], in_=zpad[:])

            # ---------------- P0/P1: x -> h strip (transposed, bf16) --------
            with tc.tile_pool(name="p0", bufs=1) as p0, \
                 tc.tile_pool(name="p0w", bufs=3) as p0w, \
                 tc.tile_pool(name="p0p", bufs=1, space="PSUM") as p0p:
                xT_a = p0.tile([128, NSH], f32)
                xT_b = p0.tile([38, NSH], f32)
                nc.vector.memset(xT_b[:], 1.0)
                for j0, jn in n_strip_chunks():
                    xc = p0w.tile([125, F_IN], f32, tag="xc")
                    nc.sync.dma_start(out=xc[:jn, :], in_=x_in[j0:j0 + jn, :])
                    tp1 = p0p.tile([P, 125], f32, tag="tp1")
                    nc.tensor.transpose(out=tp1[:, :jn], in_=xc[:jn, 0:128], identity=ident[:jn, :jn])
                    nc.scalar.activation(out=xT_a[:, j0:j0 + jn], in_=tp1[:, :jn], func=AF.Copy)
                    tp2 = p0p.tile([37, 125], f32, tag="tp2")
                    nc.tensor.transpose(out=tp2[:, :jn], in_=xc[:jn, 128:165], identity=ident[:jn, :jn])
                    nc.scalar.activation(out=xT_b[0:37, j0:j0 + jn], in_=tp2[:, :jn], func=AF.Copy)

                hT = p0.tile([D + 1, NSH], bf16)
                nc.vector.memset(hT[:], 1.0)
                for j0, jn in n_strip_chunks():
                    hp = p0p.tile([125, D], f32, tag="hp")
                    nc.tensor.matmul(out=hp[:jn, :], lhsT=xT_a[:, j0:j0 + jn], rhs=pwa[:],
                                     start=True, stop=False)
                    nc.tensor.matmul(out=hp[:jn, :], lhsT=xT_b[:, j0:j0 + jn], rhs=pwb[:],
                                     start=False, stop=True)
                    hs = p0w.tile([125, D], f32, tag="hs")
                    nc.scalar.activation(out=hs[:jn, :], in_=hp[:jn, :], func=AF.Relu)
                    ht_p = p0p.tile([D, 125], f32, tag="htp")
                    nc.tensor.transpose(out=ht_p[:, :jn], in_=hs[:jn, :], identity=ident[:jn, :jn])
                    nc.scalar.activation(out=hT[0:D, j0:j0 + jn], in_=ht_p[:, :jn], func=AF.Copy)

                nc.sync.dma_start(out=h1sh[:], in_=hT[:])
                # local a_d1 strip from hT
                for j0, jn in n_strip_chunks():
                    adp = p0p.tile([125, H1], f32, tag="adp")
                    nc.tensor.matmul(out=adp[:jn, :], lhsT=hT[:, j0:j0 + jn],
                                     rhs=w1p_bf[:, R1:R1 + 4], start=True, stop=True)
                    ads = p0w.tile([125, H1], bf16, tag="ads")
                    nc.scalar.activation(out=ads[:jn, :], in_=adp[:jn, :], func=AF.Copy)
                    nc.sync.dma_start(out=ad1_d[j0:j0 + jn, :], in_=ads[:jn, :])

            nc.gpsimd.collective_compute(
                "AllGather", OP.bypass, replica_groups=[list(range(NCORES))],
                ins=[h1sh[:]], outs=[h1full[:]])

            # ================= repeated region 1: T1 build + L1 edge ==========
            rc1 = rep_ctx()
            rc1.__enter__()
            adfull1 = cpool.tile([P, NBLK * H1], bf16)
            nc.sync.dma_start(
                out=adfull1[:].rearrange("p (b h) -> p b h", h=H1),
                in_=ad1_d[:].rearrange("(b p) h -> p b h", p=P))

            # ---------------- P3: build T1 (batched strip DMAs) --------------
            with tc.tile_pool(name="p3", bufs=2) as p3, \
                 tc.tile_pool(name="p3s", bufs=2) as p3s, \
                 tc.tile_pool(name="p3p", bufs=4, space="PSUM") as p3p:
                for s in (range(NCORES) if "tables" not in ablate else []):
                    lhT = p3.tile([D + 1, NSH], bf16, tag="lhT")
                    nc.sync.dma_start(out=lhT[:], in_=h1full[s, :, :])
                    for half in range(2):
                        ostg = p3s.tile([125, 25 * R1], bf16, tag="ostg")
                        for j in range(25):
                            j0 = (half * 25 + j) * 125
                            tp = p3p.tile([125, R1], f32, tag="tp")
                            nc.tensor.matmul(out=tp[:], lhsT=lhT[:, j0:j0 + 125],
                                             rhs=w1p_bf[:, 0:R1], start=True, stop=True)
                            nc.scalar.activation(out=ostg[:, j * R1:(j + 1) * R1],
                                                 in_=tp[:], func=AF.Copy)
                        base = 1 + s * NSH + half * 3125
                        nc.sync.dma_start(
                            out=T1[base:base + 3125, 0:R1].rearrange(
                                "(j p) c -> p j c", p=125),
                            in_=ostg[:].rearrange("p (j c) -> p j c", c=R1))

            # ---------------- P4: L1 edge phase ------------------------------
            h2T_a = cpool.tile([128, NPAD], bf16)
            h2T_b = cpool.tile([128, NPAD], bf16)
            h2T_c = cpool.tile([1, NPAD], bf16)
            nc.vector.memset(h2T_c[:], 1.0)

            with tc.tile_pool(name="p4", bufs=3) as p4, \
                 tc.tile_pool(name="p4m", bufs=2) as p4m, \
                 tc.tile_pool(name="p4p", bufs=2, space="PSUM") as p4p, \
                 tc.tile_pool(name="p4q", bufs=2, space="PSUM") as p4q:
                for b in range(NBLK):
                    gblk = p4m.tile([P, cpb1 * W1ROW], bf16, tag="gblk")
                    if "gather" not in ablate:
                        for reg, ks_, view, itile, bcols in (
                                ("lo", klo1, T1[0:I16MAX, :], il1, klo1 * 8),
                                ("hi", khi1, T1[HIBASE1:, :], ih1, khi1 * 8)):
                            c0 = 0 if reg == "lo" else klo1
                            u0 = 0
                            for u in _units(ks_):
                                nc.gpsimd.dma_gather(
                                    out_ap=gblk[:, (c0 + u0) * W1ROW:
                                                (c0 + u0 + u) * W1ROW].rearrange(
                                        "p (k w) -> p k w", w=W1ROW),
                                    in_ap=view,
                                    idxs_ap=itile[:, b * bcols + u0 * 8:
                                                  b * bcols + (u0 + u) * 8],
                                    num_idxs=u * P, num_idxs_reg=u * P,
                                    elem_size=W1ROW)
                                u0 += u
                    else:
                        nc.vector.memset(gblk[:], 0.01)

                    # on-chip one-hot masks
                    medb = p4m.tile([P, cpb1 * P], bf16, tag="medb")
                    nc.vector.tensor_tensor(
                        out=medb[:].rearrange("p (k d) -> p k d", d=P),
                        in0=mrowb1[:, b * cpb1:(b + 1) * cpb1, None].to_broadcast(
                            [P, cpb1, P]),
                        in1=iota_r[:, None, :].to_broadcast([P, cpb1, P]),
                        op=OP.is_equal)
                    derep = p4m.tile([P, cpb1 * P], bf16, tag="derep")
                    nc.sync.dma_start(out=derep[:],
                                      in_=mt1_in[b:b + 1, :].to_broadcast([P, cpb1 * P]))
                    demb = p4m.tile([P, cpb1 * P], bf16, tag="demb")
                    nc.vector.tensor_tensor(
                        out=demb[:], in0=derep[:],
                        in1=iota_c[:].to_broadcast([P, cpb1 * P]),
                        op=OP.is_equal)

                    # a_d expand: adps[e, k*4+h] via one-hot matmuls
                    adps = p4q.tile([P, cpb1 * H1], f32, tag="adps")
                    for k in range(cpb1):
                        nc.tensor.matmul(out=adps[:, k * H1:(k + 1) * H1],
                                         lhsT=demb[:, k * P:(k + 1) * P],
                                         rhs=adfull1[:, b * H1:(b + 1) * H1],
                                         start=True, stop=True)

                    # logits z = a_s + a_d ; w = exp(lrelu(z))
                    g3 = gblk[:].rearrange("p (k w) -> p k w", w=W1ROW)
                    zt = p4.tile([P, cpb1 * H1], f32, tag="zt")
                    nc.vector.tensor_tensor(
                        out=zt[:], in0=adps[:],
                        in1=g3[:, :, D:D + (H1 - 1) * C1 + 1:C1], op=OP.add)
                    zs = p4.tile([P, cpb1 * H1], f32, tag="zs")
                    nc.vector.tensor_scalar_mul(out=zs[:], in0=zt[:], scalar1=0.2)
                    nc.vector.tensor_tensor(out=zt[:], in0=zt[:], in1=zs[:], op=OP.max)
                    wt = p4.tile([P, cpb1 * H1], f32, tag="wt")
                    nc.scalar.activation(out=wt[:], in_=zt[:], func=AF.Exp)
                    wb = p4.tile([P, cpb1 * H1], bf16, tag="wb")
                    nc.vector.tensor_copy(out=wb[:], in_=wt[:])

                    # messages in place: per head, gblk[:, k, h*66:(h+1)*66] *= w
                    wb3 = wb[:].rearrange("p (k h) -> p k h", h=H1)
                    for h in range(H1):
                        nc.vector.tensor_tensor(
                            out=g3[:, :, h * C1:(h + 1) * C1],
                            in0=g3[:, :, h * C1:(h + 1) * C1],
                            in1=wb3[:, :, h:h + 1].to_broadcast([P, cpb1, C1]),
                            op=OP.mult)

                    blkps = p4p.tile([P, R1], f32, tag="blkps")
                    for k in range(cpb1):
                        nc.tensor.matmul(out=blkps[:],
                                         lhsT=medb[:, k * P:(k + 1) * P],
                                         rhs=gblk[:, k * W1ROW:k * W1ROW + R1],
                                         start=(k == 0), stop=(k == cpb1 - 1))

                    # epilogue: divide, bias, relu, transpose into h2T strips
                    den = p4.tile([P, H1], f32, tag="den")
                    nc.vector.tensor_scalar_add(
                        out=den[:],
                        in0=blkps[:].rearrange("p (h c) -> p h c", h=H1)[:, :, D + 1],
                        scalar1=1e-30)
                    rec = p4.tile([P, H1], f32, tag="rec")
                    nc.vector.reciprocal(out=rec[:], in_=den[:])
                    o1 = p4.tile([P, H1 * D], f32, tag="o1")
                    nc.vector.tensor_tensor(
                        out=o1[:].rearrange("p (h c) -> p h c", h=H1),
                        in0=blkps[:].rearrange("p (h c) -> p h c", h=H1)[:, :, 0:D],
                        in1=rec[:, :, None].to_broadcast([P, H1, D]),
                        op=OP.mult)
                    nc.vector.tensor_tensor(out=o1[:], in0=o1[:], in1=b1t[:], op=OP.add)
                    h2b = p4.tile([P, H1 * D], f32, tag="h2b")
                    nc.scalar.activation(out=h2b[:], in_=o1[:], func=AF.Relu)
                    t1p = p4q.tile([P, P], f32, tag="t1p")
                    nc.tensor.transpose(out=t1p[:], in_=h2b[:, 0:128], identity=ident[:])
                    nc.scalar.activation(out=h2T_a[:, b * P:(b + 1) * P], in_=t1p[:],
                                         func=AF.Copy)
                    t2p = p4q.tile([P, P], f32, tag="t2p")
                    nc.tensor.transpose(out=t2p[:], in_=h2b[:, 128:256], identity=ident[:])
                    nc.scalar.activation(out=h2T_b[:, b * P:(b + 1) * P], in_=t2p[:],
                                         func=AF.Copy)

            rc1.__exit__(None, None, None)
            # ---------------- P5: all-gather h2 -------------------------------
            nc.sync.dma_start(out=h2sh[0:128, :], in_=h2T_a[:])
            nc.sync.dma_start(out=h2sh[128:256, :], in_=h2T_b[:])
            nc.sync.dma_start(out=h2sh[256:257, :], in_=h2T_c[:])
            nc.gpsimd.collective_compute(
                "AllGather", OP.bypass, replica_groups=[list(range(NCORES))],
                ins=[h2sh[:]], outs=[h2full[:]])

            # ================= repeated region 2: T2 build + L2 edge ==========
            rc2 = rep_ctx()
            rc2.__enter__()
            adfull2 = cpool.tile([P, NBLK], bf16)
            with tc.tile_pool(name="p6", bufs=2) as p6, \
                 tc.tile_pool(name="p6s", bufs=2) as p6s, \
                 tc.tile_pool(name="p6p", bufs=4, space="PSUM") as p6p:
                # local a_d2 per block (from SBUF h2T tiles)
                for b in range(NBLK):
                    a2p = p6p.tile([P, 1], f32, tag="a2p")
                    nc.tensor.matmul(out=a2p[:], lhsT=h2T_a[:, b * P:(b + 1) * P],
                                     rhs=w2pa_bf[:, 66:67], start=True, stop=False)
                    nc.tensor.matmul(out=a2p[:], lhsT=h2T_b[:, b * P:(b + 1) * P],
                                     rhs=w2pb_bf[:, 66:67], start=False, stop=False)
                    nc.tensor.matmul(out=a2p[:], lhsT=h2T_c[:, b * P:(b + 1) * P],
                                     rhs=w2pc_bf[:, 66:67], start=False, stop=True)
                    nc.scalar.activation(out=adfull2[:, b:b + 1], in_=a2p[:], func=AF.Copy)

                # build T2 (batched strips; rows = padded-shard id + 1)
                for s in (range(NCORES) if "tables" not in ablate else []):
                    for n0, nn in ((0, 3200), (3200, 3072)):
                        nchk = nn // P
                        la = p6.tile([128, 3200], bf16, tag="la")
                        nc.sync.dma_start(out=la[:, :nn], in_=h2full[s, 0:128, n0:n0 + nn])
                        lb = p6.tile([128, 3200], bf16, tag="lb")
                        nc.sync.dma_start(out=lb[:, :nn], in_=h2full[s, 128:256, n0:n0 + nn])
                        lc = p6.tile([1, 3200], bf16, tag="lc")
                        nc.sync.dma_start(out=lc[:, :nn], in_=h2full[s, 256:257, n0:n0 + nn])
                        ostg = p6s.tile([128, 25 * R2], bf16, tag="ostg2")
                        for j in range(nchk):
                            tp = p6p.tile([P, R2], f32, tag="tp6")
                            nc.tensor.matmul(out=tp[:], lhsT=la[:, j * P:(j + 1) * P],
                                             rhs=w2pa_bf[:, 0:R2], start=True, stop=False)
                            nc.tensor.matmul(out=tp[:], lhsT=lb[:, j * P:(j + 1) * P],
                                             rhs=w2pb_bf[:, 0:R2], start=False, stop=False)
                            nc.tensor.matmul(out=tp[:], lhsT=lc[:, j * P:(j + 1) * P],
                                             rhs=w2pc_bf[:, 0:R2], start=False, stop=True)
                            nc.scalar.activation(out=ostg[:, j * R2:(j + 1) * R2],
                                                 in_=tp[:], func=AF.Copy)
                        base = 1 + s * NPAD + n0
                        nc.sync.dma_start(
                            out=T2[base:base + nn, 0:R2].rearrange(
                                "(j p) c -> p j c", p=P),
                            in_=ostg[:, 0:nchk * R2].rearrange("p (j c) -> p j c", c=R2))

            # ---------------- P7: L2 edge phase + classifier ------------------
            h3T = cpool.tile([D + 1, NPAD], bf16)
            clsp_bf = cpool.tile([D + 1, 1], bf16)
            nc.vector.tensor_copy(out=clsp_bf[:], in_=clsp[:])
            nc.vector.memset(h3T[:], 1.0)

            with tc.tile_pool(name="p7", bufs=3) as p7, \
                 tc.tile_pool(name="p7m", bufs=2) as p7m, \
                 tc.tile_pool(name="p7p", bufs=2, space="PSUM") as p7p, \
                 tc.tile_pool(name="p7q", bufs=2, space="PSUM") as p7q:
                for b in range(NBLK):
                    gblk2 = p7m.tile([P, cpb2 * W2ROW], bf16, tag="gblk2")
                    if "gather" not in ablate:
                        for reg, ks_, view, itile, bcols in (
                                ("lo", klo2, T2[0:I16MAX, :], il2, klo2 * 8),
                                ("hi", khi2, T2[HIBASE2:, :], ih2, khi2 * 8)):
                            c0 = 0 if reg == "lo" else klo2
                            u0 = 0
                            for u in _units(ks_):
                                nc.gpsimd.dma_gather(
                                    out_ap=gblk2[:, (c0 + u0) * W2ROW:
                                                 (c0 + u0 + u) * W2ROW].rearrange(
                                        "p (k w) -> p k w", w=W2ROW),
                                    in_ap=view,
                                    idxs_ap=itile[:, b * bcols + u0 * 8:
                                                  b * bcols + (u0 + u) * 8],
                                    num_idxs=u * P, num_idxs_reg=u * P,
                                    elem_size=W2ROW)
                                u0 += u
                    else:
                        nc.vector.memset(gblk2[:], 0.01)

                    medb2 = p7m.tile([P, cpb2 * P], bf16, tag="medb2")
                    nc.vector.tensor_tensor(
                        out=medb2[:].rearrange("p (k d) -> p k d", d=P),
                        in0=mrowb2[:, b * cpb2:(b + 1) * cpb2, None].to_broadcast(
                            [P, cpb2, P]),
                        in1=iota_r[:, None, :].to_broadcast([P, cpb2, P]),
                        op=OP.is_equal)
                    derep2 = p7m.tile([P, cpb2 * P], bf16, tag="derep2")
                    nc.sync.dma_start(out=derep2[:],
                                      in_=mt2_in[b:b + 1, :].to_broadcast([P, cpb2 * P]))
                    demb2 = p7m.tile([P, cpb2 * P], bf16, tag="demb2")
                    nc.vector.tensor_tensor(
                        out=demb2[:], in0=derep2[:],
                        in1=iota_c[:].to_broadcast([P, cpb2 * P]),
                        op=OP.is_equal)

                    adps2 = p7q.tile([P, cpb2], f32, tag="adps2")
                    for k in range(cpb2):
                        nc.tensor.matmul(out=adps2[:, k:k + 1],
                                         lhsT=demb2[:, k * P:(k + 1) * P],
                                         rhs=adfull2[:, b:b + 1],
                                         start=True, stop=True)

                    g23 = gblk2[:].rearrange("p (k w) -> p k w", w=W2ROW)
                    zt2 = p7.tile([P, cpb2], f32, tag="zt2")
                    nc.vector.tensor_tensor(
                        out=zt2[:], in0=adps2[:], in1=g23[:, :, D], op=OP.add)
                    zs2 = p7.tile([P, cpb2], f32, tag="zs2")
                    nc.vector.tensor_scalar_mul(out=zs2[:], in0=zt2[:], scalar1=0.2)
                    nc.vector.tensor_tensor(out=zt2[:], in0=zt2[:], in1=zs2[:], op=OP.max)
                    wt2 = p7.tile([P, cpb2], f32, tag="wt2")
                    nc.scalar.activation(out=wt2[:], in_=zt2[:], func=AF.Exp)
                    wb2 = p7.tile([P, cpb2], bf16, tag="wb2")
                    nc.vector.tensor_copy(out=wb2[:], in_=wt2[:])

                    nc.vector.tensor_tensor(
                        out=g23[:, :, 0:R2], in0=g23[:, :, 0:R2],
                        in1=wb2[:, :, None].to_broadcast([P, cpb2, R2]),
                        op=OP.mult)

                    blkps2 = p7p.tile([P, R2], f32, tag="blkps2")
                    for k in range(cpb2):
                        nc.tensor.matmul(out=blkps2[:],
                                         lhsT=medb2[:, k * P:(k + 1) * P],
                                         rhs=gblk2[:, k * W2ROW:k * W2ROW + R2],
                                         start=(k == 0), stop=(k == cpb2 - 1))

                    den2 = p7.tile([P, 1], f32, tag="den2")
                    nc.vector.tensor_scalar_add(out=den2[:], in0=blkps2[:, D + 1:D + 2],
                                                scalar1=1e-30)
                    rec2 = p7.tile([P, 1], f32, tag="rec2")
                    nc.vector.reciprocal(out=rec2[:], in_=den2[:])
                    o2 = p7.tile([P, D], f32, tag="o2")
                    nc.vector.tensor_scalar(out=o2[:], in0=blkps2[:, 0:D],
                                            scalar1=rec2[:], scalar2=None, op0=OP.mult)
                    nc.vector.tensor_tensor(out=o2[:], in0=o2[:], in1=b2t[:], op=OP.add)
                    h3b = p7.tile([P, D], f32, tag="h3b")
                    nc.scalar.activation(out=h3b[:], in_=o2[:], func=AF.Relu)
                    t3p = p7q.tile([D, P], f32, tag="t3p")
                    nc.tensor.transpose(out=t3p[:], in_=h3b[:], identity=ident[:])
                    nc.scalar.activation(out=h3T[0:D, b * P:(b + 1) * P], in_=t3p[:],
                                         func=AF.Copy)

                # classifier: y = h3 @ cls_W + cls_b
                yt = cpool.tile([P, NBLK], f32)
                for b in range(NBLK):
                    yp = p7q.tile([P, 1], f32, tag="yp")
                    nc.tensor.matmul(out=yp[:], lhsT=h3T[:, b * P:(b + 1) * P],
                                     rhs=clsp_bf[:], start=True, stop=True)
                    nc.vector.tensor_copy(out=yt[:, b:b + 1], in_=yp[:])
                nc.sync.dma_start(out=y_out[:].rearrange("(b p) -> p b", p=P), in_=yt[:])
            rc2.__exit__(None, None, None)

    nc.compile()
    return nc


# ----------------------------------------------------------------------------
# Entry point
# ----------------------------------------------------------------------------

_CACHE = {}


def kernel(**inputs):
    wts = prep_weights(inputs)
    cores, ks = prep_edges(inputs["edge_index"])
    x = np.asarray(inputs["x"], np.float32)

    reps = int(inputs.pop("_reps", 1)) if "_reps" in inputs else 1
    ablate = tuple(inputs.pop("_ablate", ()))
    key = ("prog", ks, reps, ablate)
    if key not in _CACHE:
        _CACHE[key] = build_program(ks, reps, ablate)
    nc = _CACHE[key]

    in_maps = []
    for c in range(NCORES):
        m = {"x_strip": np.ascontiguousarray(x[c * NSH:(c + 1) * NSH])}
        m.update(cores[c])
        m.update(wts)
        in_maps.append(m)

    res = run_bass_kernel_spmd(nc, in_maps, list(range(NCORES)))
    y = np.concatenate([res.results[c]["y"][:NSH] for c in range(NCORES)])
    return y.astype(np.float32)
